# revision 39
# baseline (speedup 1.0000x reference)
"""Trainium2 Bass kernel for zonotope AbstractRelu (nn_AbstractRelu_76751065579631).

Problem: x [E=512, D1=4096, D2=16] f32. Per neuron column n (N = D1*D2 = 65536):
    sum_abs[n] = sum_{e>=1} |x[e, n]|
    lb = x[0] - sum_abs ; ub = x[0] + sum_abs
    crossing = lb <= 0 & ub >= 0 ; dead = ub <= 0
    alpha = crossing ? 1 - lb : 1
    out[0]   = dead ? 0 : (crossing ? alpha*(x[0] - lb/2) : x[0])
    out[1:]  = dead ? 0 : (crossing ? alpha : 1) * x[1:]

Sharding: neuron columns split contiguously across 8 cores (8192 each), no
communication. On-chip layout keeps error terms on partitions (4 blocks of
128) so every HBM DMA moves >=8KiB contiguous per partition. The
cross-partition reduction over error terms runs on the (otherwise idle)
TensorE as a matmul with a ones vector; the per-neuron scale is broadcast
back across partitions with a K=1 ones matmul into PSUM.
"""

import os

import numpy as np

E = 512
D1 = 4096
D2 = 16
N = D1 * D2          # 65536 neurons
NCORES = 8
COLS = N // NCORES   # 8192 neuron columns per core
W = 2048             # chunk width

LAST_EXEC_TIME_NS = None

_CACHE = {}


def _emit(tc, out_ap, x_ap, W):
    import concourse.mybir as mybir

    nc = tc.nc
    f32 = mybir.dt.float32
    Alu = mybir.AluOpType
    Act = mybir.ActivationFunctionType

    e_total, cols = x_ap.shape
    NB = e_total // 128          # e-blocks of 128 partitions
    widths = [W] * (cols // W)

    # DRAM views [NB, 128, cols]: per-block DMAs keep an unambiguous
    # partition<->partition pairing (always 128 partitions, so the HWDGE
    # splitter spreads descriptors across all 16 SDMA engines).
    x_blk = x_ap.rearrange("(b p) n -> b p n", p=128)
    o_blk = out_ap.rearrange("(b p) n -> b p n", p=128)

    with (
        tc.tile_pool(name="const", bufs=1) as const_pool,
        tc.tile_pool(name="x", bufs=3) as x_pool,
        tc.tile_pool(name="work", bufs=2) as work_pool,
        tc.tile_pool(name="part", bufs=2) as part_pool,
        tc.tile_pool(name="rows", bufs=1) as row_pool,
        tc.tile_pool(name="small", bufs=3) as small_pool,
        tc.tile_pool(name="psum", bufs=1, space="PSUM") as psum_pool,
    ):
        ones_col = const_pool.tile([128, 1], f32, tag="ones_col")
        nc.vector.memset(ones_col[:], 1.0)
        ones_row = const_pool.tile([1, 128], f32, tag="ones_row")
        nc.vector.memset(ones_row[:], 1.0)

        def front(cs, W):
            """Load + |x| + partial adds + reduce + repartition issue."""
            NS = W // 512
            WP = W // 128
            st = {"cs": cs, "W": W, "WP": WP, "NS": NS}

            xt = x_pool.tile([128, NB * W], f32, tag="x")
            blk = [xt[:, W * b:W * (b + 1)] for b in range(NB)]
            for b in range(NB):
                nc.sync.dma_start(out=blk[b], in_=x_blk[b, :, cs:cs + W])
            # center row straight from DRAM: starts before any big load lands
            c_t = small_pool.tile([128, WP], f32, tag="ct")
            nc.gpsimd.dma_start(out=c_t[:], in_=x_blk[0, 0:1, cs:cs + W])
            absc = small_pool.tile([128, WP], f32, tag="absc")
            nc.scalar.activation(absc[:], c_t[:], Act.Abs)

            # |x| for blocks (0,1) into the halves of a01, (2,3) into a23;
            # one fused pair-add + one final add (all DVE; GpSimd kept off
            # the shared DVE/GpSimd SBUF port)
            a01 = work_pool.tile([128, 2 * W], f32, tag="a01")
            a23 = work_pool.tile([128, 2 * W], f32, tag="a23")
            for b in range(NB):
                dst = (a01 if b < 2 else a23)[:, W * (b % 2):W * (b % 2 + 1)]
                nc.scalar.activation(dst, blk[b], Act.Abs)
            nc.vector.tensor_add(a01[:], a01[:], a23[:])
            ut = part_pool.tile([128, W], f32, tag="u")
            u = ut[:]
            nc.vector.tensor_add(u, a01[:, 0:W], a01[:, W:2 * W])

            # column sums of u via TensorE (ones^T @ u), one matmul per bank
            psum_s = psum_pool.tile([1, W], f32, tag="s")
            for s in range(NS):
                nc.tensor.matmul(
                    psum_s[0:1, 512 * s:512 * (s + 1)],
                    lhsT=ones_col[:],
                    rhs=u[:, 512 * s:512 * (s + 1)],
                    start=True,
                    stop=True,
                )
            s_row = row_pool.tile([1, W], f32, tag="srow")
            nc.scalar.copy(s_row[:], psum_s[:])          # PSUM -> SBUF
            s_t = small_pool.tile([128, WP], f32, tag="st")
            nc.gpsimd.dma_start(out=s_t[:], in_=s_row[:])  # [1,W] -> [128,WP]

            st.update(xt=xt, blk=blk, c_t=c_t, absc=absc, s_t=s_t)
            return st

        def back(st):
            """Per-neuron math, broadcast, fused multiply, stores."""
            cs, W, WP, NS = st["cs"], st["W"], st["WP"], st["NS"]
            xt, blk, c_t, absc, s_t = (
                st["xt"], st["blk"], st["c_t"], st["absc"], st["s_t"]
            )
            # serr_n = |c| - sum_all|x| = -sum_{e>=1}|x|
            serr_n = small_pool.tile([128, WP], f32, tag="serrn")
            nc.vector.tensor_sub(serr_n[:], absc[:], s_t[:])
            lb = small_pool.tile([128, WP], f32, tag="lb")
            nc.vector.tensor_add(lb[:], c_t[:], serr_n[:])
            ub = small_pool.tile([128, WP], f32, tag="ub")
            nc.vector.tensor_sub(ub[:], c_t[:], serr_n[:])
            min0 = small_pool.tile([128, WP], f32, tag="min0")
            nc.vector.tensor_scalar_min(min0[:], lb[:], 0.0)
            alpha = small_pool.tile([128, WP], f32, tag="alpha")
            nc.vector.tensor_scalar(alpha[:], min0[:], -1.0, 1.0, Alu.mult, Alu.add)
            gt = small_pool.tile([128, WP], f32, tag="gt")
            nc.vector.tensor_scalar(gt[:], ub[:], 0.0, None, Alu.is_gt)
            scale = small_pool.tile([128, WP], f32, tag="scale")
            nc.vector.tensor_mul(scale[:], alpha[:], gt[:])
            t1 = small_pool.tile([128, WP], f32, tag="t1")
            nc.vector.scalar_tensor_tensor(
                t1[:], in0=min0[:], scalar=-0.5, in1=c_t[:],
                op0=Alu.mult, op1=Alu.add,
            )
            cnew = small_pool.tile([128, WP], f32, tag="cnew")
            nc.vector.tensor_mul(cnew[:], t1[:], scale[:])

            scale_row = row_pool.tile([1, W], f32, tag="scrow")
            nc.gpsimd.dma_start(out=scale_row[:], in_=scale[:])

            # broadcast scale across partitions (TensorE K=1 ones matmul into
            # PSUM), then one fused in-place multiply over all NB blocks
            # (stride-0 repeat along the block axis)
            psum_b = psum_pool.tile([128, W], f32, tag="b")
            for s in range(NS):
                nc.tensor.matmul(
                    psum_b[:, 512 * s:512 * (s + 1)],
                    lhsT=ones_row[:],
                    rhs=scale_row[0:1, 512 * s:512 * (s + 1)],
                    start=True,
                    stop=True,
                )
            xt3 = xt[:].rearrange("p (b n) -> p b n", b=NB)
            nc.vector.tensor_mul(
                xt3, xt3, psum_b[:, None, :].broadcast_to([128, NB, W])
            )
            # patch the new center row over the (garbage) scaled partition 0;
            # store block 0 last so the patch doesn't gate the other stores
            nc.gpsimd.dma_start(out=xt[0:1, 0:W], in_=cnew[:])
            for b in list(range(1, NB)) + [0]:
                nc.sync.dma_start(out=o_blk[b, :, cs:cs + W], in_=blk[b])

        starts = [sum(widths[:i]) for i in range(len(widths))]
        for k, Wk in enumerate(widths):
            back(front(starts[k], Wk))


def build(cols=COLS, e_total=E, w=W):
    """Build + compile the per-core Bass program (cached)."""
    key = (cols, e_total, w)
    if key in _CACHE:
        return _CACHE[key]

    from concourse import bacc
    import concourse.mybir as mybir
    from concourse.tile import TileContext

    nc = bacc.Bacc("TRN2", target_bir_lowering=False, debug=False,
                   num_devices=NCORES)
    x_ap = nc.dram_tensor("x", [e_total, cols], mybir.dt.float32,
                          kind="ExternalInput").ap()
    out_ap = nc.dram_tensor("o", [e_total, cols], mybir.dt.float32,
                            kind="ExternalOutput").ap()
    with TileContext(nc) as tc:
        _emit(tc, out_ap, x_ap, w)
    nc.compile()
    _CACHE[key] = nc
    return nc


def _ensure_ntff_hook():
    """Install the axon NTFF profile hook when the image's antenv lacks it.

    The agent image's ``antenv`` has no ``axon_hooks`` module, so
    ``run_bass_kernel_spmd(trace=True)`` under axon would silently skip
    tracing. Inject an equivalent module and wire it to the ctypes-based
    profile hook from ``trn_agent_boot``.
    """
    import sys
    import types

    try:
        from antenv.axon_hooks import get_axon_ntff_profile_hook  # noqa: F401
        return
    except ImportError:
        pass

    mod = types.ModuleType("antenv.axon_hooks")
    mod._hook = None

    def set_axon_ntff_profile_hook(h):
        mod._hook = h

    def get_axon_ntff_profile_hook():
        return mod._hook

    mod.set_axon_ntff_profile_hook = set_axon_ntff_profile_hook
    mod.get_axon_ntff_profile_hook = get_axon_ntff_profile_hook
    sys.modules["antenv.axon_hooks"] = mod
    import antenv

    antenv.axon_hooks = mod
    try:
        from trn_agent_boot.trn_boot import _ntff_profile_via_ctypes

        set_axon_ntff_profile_hook(
            _ntff_profile_via_ctypes("/opt/axon/libaxon_pjrt.so")
        )
    except Exception:
        pass


def kernel(x):
    global LAST_EXEC_TIME_NS
    from concourse import bass_utils

    nc = build()
    xf = np.ascontiguousarray(np.asarray(x, dtype=np.float32).reshape(E, N))
    in_maps = [
        {"x": np.ascontiguousarray(xf[:, c * COLS:(c + 1) * COLS])}
        for c in range(NCORES)
    ]
    trace = bool(int(os.environ.get("KERNEL_TRACE", "0")))
    if trace:
        _ensure_ntff_hook()
        # Sandboxed container: keep profile artifacts local.
        bass_utils.upload_artifacts = lambda tmpdir: tmpdir
    res = bass_utils.run_bass_kernel_spmd(
        nc, in_maps, core_ids=list(range(NCORES)), trace=trace
    )
    LAST_EXEC_TIME_NS = res.exec_time_ns
    out = np.concatenate([res.results[c]["o"] for c in range(NCORES)], axis=1)
    return out.reshape(E, D1, D2)


# revision 40
# speedup vs baseline: 1.1435x; 1.1435x over previous
"""Trainium2 Bass kernel for zonotope AbstractRelu (nn_AbstractRelu_76751065579631).

Problem: x [E=512, D1=4096, D2=16] f32. Per neuron column n (N = D1*D2 = 65536):
    sum_abs[n] = sum_{e>=1} |x[e, n]|
    lb = x[0] - sum_abs ; ub = x[0] + sum_abs
    crossing = lb <= 0 & ub >= 0 ; dead = ub <= 0
    alpha = crossing ? 1 - lb : 1
    out[0]   = dead ? 0 : (crossing ? alpha*(x[0] - lb/2) : x[0])
    out[1:]  = dead ? 0 : (crossing ? alpha : 1) * x[1:]

Sharding: neuron columns split contiguously across 8 cores (8192 each), no
communication. On-chip layout keeps error terms on partitions (4 blocks of
128) so every HBM DMA moves >=8KiB contiguous per partition. The
cross-partition reduction over error terms runs on the (otherwise idle)
TensorE as a matmul with a ones vector; the per-neuron scale is broadcast
back across partitions with a K=1 ones matmul into PSUM.
"""

import os

import numpy as np

E = 512
D1 = 4096
D2 = 16
N = D1 * D2          # 65536 neurons
NCORES = 8
COLS = N // NCORES   # 8192 neuron columns per core
W = 2048             # chunk width

LAST_EXEC_TIME_NS = None

_CACHE = {}


def _emit(tc, out_ap, x_ap, W):
    import concourse.mybir as mybir

    nc = tc.nc
    f32 = mybir.dt.float32
    Alu = mybir.AluOpType
    Act = mybir.ActivationFunctionType

    e_total, cols = x_ap.shape
    NB = e_total // 128          # e-blocks of 128 partitions
    widths = [W] * (cols // W)

    # DRAM views [NB, 128, cols]: per-block DMAs keep an unambiguous
    # partition<->partition pairing (always 128 partitions, so the HWDGE
    # splitter spreads descriptors across all 16 SDMA engines).
    x_blk = x_ap.rearrange("(b p) n -> b p n", p=128)
    o_blk = out_ap.rearrange("(b p) n -> b p n", p=128)

    with (
        tc.tile_pool(name="const", bufs=1) as const_pool,
        tc.tile_pool(name="x", bufs=3) as x_pool,
        tc.tile_pool(name="work", bufs=2) as work_pool,
        tc.tile_pool(name="part", bufs=2) as part_pool,
        tc.tile_pool(name="rows", bufs=1) as row_pool,
        tc.tile_pool(name="small", bufs=3) as small_pool,
        tc.tile_pool(name="psum", bufs=1, space="PSUM") as psum_pool,
    ):
        ones_col = const_pool.tile([128, 1], f32, tag="ones_col")
        nc.vector.memset(ones_col[:], 1.0)
        ones_row = const_pool.tile([1, 128], f32, tag="ones_row")
        nc.vector.memset(ones_row[:], 1.0)

        def front(cs, W):
            """Load + |x| + partial adds + reduce + repartition issue."""
            NS = W // 512
            WP = W // 128
            st = {"cs": cs, "W": W, "WP": WP, "NS": NS}

            xt = x_pool.tile([128, NB * W], f32, tag="x")
            blk = [xt[:, W * b:W * (b + 1)] for b in range(NB)]
            for b in range(NB):
                nc.sync.dma_start(out=blk[b], in_=x_blk[b, :, cs:cs + W])
            # center row and |center| depend only on block 0 -- extract early
            c_t = small_pool.tile([128, WP], f32, tag="ct")
            nc.gpsimd.dma_start(out=c_t[:], in_=xt[0:1, 0:W])
            absc = small_pool.tile([128, WP], f32, tag="absc")
            nc.scalar.activation(absc[:], c_t[:], Act.Abs)

            # |x| for blocks (0,1) into the halves of a01, (2,3) into a23;
            # one fused pair-add + one final add (all DVE; GpSimd kept off
            # the shared DVE/GpSimd SBUF port)
            a01 = work_pool.tile([128, 2 * W], f32, tag="a01")
            a23 = work_pool.tile([128, 2 * W], f32, tag="a23")
            for b in range(NB):
                dst = (a01 if b < 2 else a23)[:, W * (b % 2):W * (b % 2 + 1)]
                nc.scalar.activation(dst, blk[b], Act.Abs)
            nc.vector.tensor_add(a01[:], a01[:], a23[:])
            ut = part_pool.tile([128, W], f32, tag="u")
            u = ut[:]
            nc.vector.tensor_add(u, a01[:, 0:W], a01[:, W:2 * W])

            # column sums of u via TensorE (ones^T @ u), one matmul per bank
            psum_s = psum_pool.tile([1, W], f32, tag="s")
            for s in range(NS):
                nc.tensor.matmul(
                    psum_s[0:1, 512 * s:512 * (s + 1)],
                    lhsT=ones_col[:],
                    rhs=u[:, 512 * s:512 * (s + 1)],
                    start=True,
                    stop=True,
                )
            s_row = row_pool.tile([1, W], f32, tag="srow")
            nc.scalar.copy(s_row[:], psum_s[:])          # PSUM -> SBUF
            s_t = small_pool.tile([128, WP], f32, tag="st")
            nc.gpsimd.dma_start(out=s_t[:], in_=s_row[:])  # [1,W] -> [128,WP]

            st.update(xt=xt, blk=blk, c_t=c_t, absc=absc, s_t=s_t)
            return st

        def back(st):
            """Per-neuron math, broadcast, fused multiply, stores."""
            cs, W, WP, NS = st["cs"], st["W"], st["WP"], st["NS"]
            xt, blk, c_t, absc, s_t = (
                st["xt"], st["blk"], st["c_t"], st["absc"], st["s_t"]
            )
            # serr_n = |c| - sum_all|x| = -sum_{e>=1}|x|
            serr_n = small_pool.tile([128, WP], f32, tag="serrn")
            nc.vector.tensor_sub(serr_n[:], absc[:], s_t[:])
            lb = small_pool.tile([128, WP], f32, tag="lb")
            nc.vector.tensor_add(lb[:], c_t[:], serr_n[:])
            ub = small_pool.tile([128, WP], f32, tag="ub")
            nc.vector.tensor_sub(ub[:], c_t[:], serr_n[:])
            min0 = small_pool.tile([128, WP], f32, tag="min0")
            nc.vector.tensor_scalar_min(min0[:], lb[:], 0.0)
            alpha = small_pool.tile([128, WP], f32, tag="alpha")
            nc.vector.tensor_scalar(alpha[:], min0[:], -1.0, 1.0, Alu.mult, Alu.add)
            gt = small_pool.tile([128, WP], f32, tag="gt")
            nc.vector.tensor_scalar(gt[:], ub[:], 0.0, None, Alu.is_gt)
            scale = small_pool.tile([128, WP], f32, tag="scale")
            nc.vector.tensor_mul(scale[:], alpha[:], gt[:])
            t1 = small_pool.tile([128, WP], f32, tag="t1")
            nc.vector.scalar_tensor_tensor(
                t1[:], in0=min0[:], scalar=-0.5, in1=c_t[:],
                op0=Alu.mult, op1=Alu.add,
            )
            cnew = small_pool.tile([128, WP], f32, tag="cnew")
            nc.vector.tensor_mul(cnew[:], t1[:], scale[:])

            scale_row = row_pool.tile([1, W], f32, tag="scrow")
            nc.gpsimd.dma_start(out=scale_row[:], in_=scale[:])

            # broadcast scale across partitions (TensorE K=1 ones matmul into
            # PSUM), then one fused in-place multiply over all NB blocks
            # (stride-0 repeat along the block axis)
            psum_b = psum_pool.tile([128, W], f32, tag="b")
            for s in range(NS):
                nc.tensor.matmul(
                    psum_b[:, 512 * s:512 * (s + 1)],
                    lhsT=ones_row[:],
                    rhs=scale_row[0:1, 512 * s:512 * (s + 1)],
                    start=True,
                    stop=True,
                )
            xt3 = xt[:].rearrange("p (b n) -> p b n", b=NB)
            nc.vector.tensor_mul(
                xt3, xt3, psum_b[:, None, :].broadcast_to([128, NB, W])
            )
            # patch the new center row over the (garbage) scaled partition 0
            nc.gpsimd.dma_start(out=xt[0:1, 0:W], in_=cnew[:])
            for b in range(NB):
                nc.sync.dma_start(out=o_blk[b, :, cs:cs + W], in_=blk[b])

        starts = [sum(widths[:i]) for i in range(len(widths))]
        for k, Wk in enumerate(widths):
            back(front(starts[k], Wk))


def build(cols=COLS, e_total=E, w=W):
    """Build + compile the per-core Bass program (cached)."""
    key = (cols, e_total, w)
    if key in _CACHE:
        return _CACHE[key]

    from concourse import bacc
    import concourse.mybir as mybir
    from concourse.tile import TileContext

    nc = bacc.Bacc("TRN2", target_bir_lowering=False, debug=False,
                   num_devices=NCORES)
    x_ap = nc.dram_tensor("x", [e_total, cols], mybir.dt.float32,
                          kind="ExternalInput").ap()
    out_ap = nc.dram_tensor("o", [e_total, cols], mybir.dt.float32,
                            kind="ExternalOutput").ap()
    with TileContext(nc) as tc:
        _emit(tc, out_ap, x_ap, w)
    nc.compile()
    _CACHE[key] = nc
    return nc


def _ensure_ntff_hook():
    """Install the axon NTFF profile hook when the image's antenv lacks it.

    The agent image's ``antenv`` has no ``axon_hooks`` module, so
    ``run_bass_kernel_spmd(trace=True)`` under axon would silently skip
    tracing. Inject an equivalent module and wire it to the ctypes-based
    profile hook from ``trn_agent_boot``.
    """
    import sys
    import types

    try:
        from antenv.axon_hooks import get_axon_ntff_profile_hook  # noqa: F401
        return
    except ImportError:
        pass

    mod = types.ModuleType("antenv.axon_hooks")
    mod._hook = None

    def set_axon_ntff_profile_hook(h):
        mod._hook = h

    def get_axon_ntff_profile_hook():
        return mod._hook

    mod.set_axon_ntff_profile_hook = set_axon_ntff_profile_hook
    mod.get_axon_ntff_profile_hook = get_axon_ntff_profile_hook
    sys.modules["antenv.axon_hooks"] = mod
    import antenv

    antenv.axon_hooks = mod
    try:
        from trn_agent_boot.trn_boot import _ntff_profile_via_ctypes

        set_axon_ntff_profile_hook(
            _ntff_profile_via_ctypes("/opt/axon/libaxon_pjrt.so")
        )
    except Exception:
        pass


def kernel(x):
    global LAST_EXEC_TIME_NS
    from concourse import bass_utils

    nc = build()
    xf = np.ascontiguousarray(np.asarray(x, dtype=np.float32).reshape(E, N))
    in_maps = [
        {"x": np.ascontiguousarray(xf[:, c * COLS:(c + 1) * COLS])}
        for c in range(NCORES)
    ]
    trace = bool(int(os.environ.get("KERNEL_TRACE", "0")))
    if trace:
        _ensure_ntff_hook()
        # Sandboxed container: keep profile artifacts local.
        bass_utils.upload_artifacts = lambda tmpdir: tmpdir
    res = bass_utils.run_bass_kernel_spmd(
        nc, in_maps, core_ids=list(range(NCORES)), trace=trace
    )
    LAST_EXEC_TIME_NS = res.exec_time_ns
    out = np.concatenate([res.results[c]["o"] for c in range(NCORES)], axis=1)
    return out.reshape(E, D1, D2)
